# revision 1
# baseline (speedup 1.0000x reference)
"""ChainCRF Viterbi decode kernel for Trainium2 (8 NeuronCores, data parallel).

Problem: x [1024, 1024, 48] f32, transition [48, 48] f32.
Reference: per-sequence Viterbi (max-plus DP over T=1024 steps, C=48 tags,
backtrack, one-hot output [B, T, C]).

Sharding: batch 1024 -> 8 cores x 128 sequences; partition = sequence.

Forward step t (all vector engine):
  s[b,k,j]   = delta[b,j] + trans[j,k]        (stride-0 broadcast add)
  nd[b,k]    = max_j s[b,k,j]                 (tensor_reduce axis=X)
  ind[b,k,j] = (s == nd)                      (stride-0 is_equal)
  msk[b,k,j] = ind * (64 - j)                 (winners >= 17, losers 0)
  bp[b,k]    = max_j msk  -> uint8 SBUF buffer (encodes 64 - argmax_j,
               smallest index wins ties, matching jnp.argmax)
  delta'[b,k] = nd[b,k] + x[b,t,k]
Backward replays bp with one-hot selection (path encoded as 64 - j):
  oh[b,k] = (jenc[k] == path[b])  -> IS the output one-hot tile for step t
  path'   = sum_k oh[b,k] * bp_t[b,k]
Exact w.r.t. the reference incl. f32 arithmetic order and argmax tie-breaks.

DMA/queue budgeting: HW instructions carry at most ONE semaphore wait.  A
DMA needs a data wait (cross-engine dep) plus a queue-ring wait on queue
reuse, so every DMA with a data dependency must land on a virgin queue of
its engine.  Budget: 8 output stores -> the 8 SP queues; 11 recycled-slot
x loads -> 8 ACT queues + 3 spare SW queues; constants + first loads
(no data deps) -> leading SW queues.  DVE consumers take the DMA
completion wait through a dedicated in-place "absorb" copy.
"""
import sys

sys.path.insert(0, "/opt/trn_rl_repo")

from contextlib import ExitStack

import numpy as np

import concourse.bass as bass
import concourse.tile as tile
from concourse import mybir
from concourse.bass_utils import run_bass_kernel_spmd
from concourse.tile_rust import add_dep_helper

B, T, C = 1024, 1024, 48
NCORES = 8
PB = B // NCORES   # 128 sequences per core
CL = 128           # x-load chunk (time steps); 8 loads
CS = 256           # output-store chunk; 4 stores
F32 = mybir.dt.float32
BF16 = mybir.dt.bfloat16
U8 = mybir.dt.uint8


def build_kernel(nsteps=T):
    nc = bass.Bass("TRN2", num_devices=NCORES)
    x_d = nc.dram_tensor("x", [PB, T, C], F32, kind="ExternalInput").ap()
    tkj_d = nc.dram_tensor("tkj", [PB, C * C], F32, kind="ExternalInput").ap()
    jenc_d = nc.dram_tensor("jenc", [PB, C], F32, kind="ExternalInput").ap()
    jencf_d = nc.dram_tensor("jencf", [PB, C * C], BF16,
                             kind="ExternalInput").ap()
    out_d = nc.dram_tensor("out", [PB, T, C], F32, kind="ExternalOutput").ap()

    with tile.TileContext(nc) as tc, ExitStack() as ctx:
        const = ctx.enter_context(tc.tile_pool(name="const", bufs=1))
        xin = ctx.enter_context(tc.tile_pool(name="xin", bufs=2))
        ohout = ctx.enter_context(tc.tile_pool(name="ohout", bufs=1))
        state = ctx.enter_context(tc.tile_pool(name="state", bufs=2))
        big = ctx.enter_context(tc.tile_pool(name="big", bufs=2))

        dma_insts = []

        def dma(eng, out, in_):
            dma_insts.append(eng.dma_start(out, in_))

        def absorb(ap):
            # Absorb a DMA completion wait into a standalone in-place DVE
            # copy so compute ops never need a second sem wait.
            nc.vector.tensor_copy(ap, ap)

        tkj = const.tile([PB, C * C], F32)
        jenc = const.tile([PB, C], F32)   # jenc[k] = 64 - k
        dma(nc.gpsimd, tkj[:], tkj_d[:])           # SW q0, fresh slot
        absorb(tkj[:, 0:1])
        dma(nc.gpsimd, jenc[:], jenc_d[:])         # SW q1, fresh slot
        absorb(jenc[:, 0:1])
        jencf = const.tile([PB, C * C], BF16)      # jenc replicated k-major
        dma(nc.gpsimd, jencf[:], jencf_d[:])       # SW q2, fresh slot
        absorb(jencf[:, 0:1])
        tkj3 = tkj[:].rearrange("p (k j) -> p k j", k=C)

        # backpointer buffer, SBUF-resident uint8: 48 KB/partition
        bpbuf = const.tile([PB, T * C], U8)

        # ---------------- forward ----------------
        def load_chunk(ci):
            # Queue plan (each physical queue used at most once so no DMA
            # ever needs both a ring wait and a data wait):
            #   SWDGE q0,q1,q2: tkj, jenc, jencf;  q3..q7: loads ci 0..4
            #   HWDGE (all via SP) q0..q2: loads ci 5..7;  q3..q6: stores
            t0 = ci * CL
            sz = min(CL, nsteps - t0)
            xc = xin.tile([PB, CL * C], F32, tag="xc")
            eng = nc.gpsimd if ci <= 4 else nc.sync
            dma(eng, xc[:, 0:sz * C], x_d[:, t0:t0 + sz, :])
            # full-slot in-place copy: absorbs the DMA completion for DVE
            # consumers AND makes DVE the last writer of every byte so the
            # slot's next DMA needs only a DVE wait.
            absorb(xc[:, 0:sz * C])
            return xc

        delta = state.tile([PB, C], F32, tag="delta")
        xc = load_chunk(0)
        nc.vector.tensor_copy(delta[:], xc[:, 0:C])

        for t in range(1, nsteps):
            if t % CL == 0:
                xc = load_chunk(t // CL)

            s = big.tile([PB, C * C], F32, tag="s")
            s3 = s[:].rearrange("p (k j) -> p k j", k=C)
            nc.vector.tensor_tensor(
                s3, delta[:].unsqueeze(1).broadcast_to([PB, C, C]), tkj3,
                mybir.AluOpType.add)

            nd = state.tile([PB, C], F32, tag="nd")
            nc.vector.tensor_reduce(nd[:], s3, axis=mybir.AxisListType.X,
                                    op=mybir.AluOpType.max)

            # winner mask in bf16: 0.0/1.0 and the 17..64 index codes are
            # all bf16-exact, and the all-bf16 multiply runs in the DVE 2x
            # perf mode (the f32 compare itself is unchanged).
            ind = big.tile([PB, C * C], BF16, tag="ind")
            ind3 = ind[:].rearrange("p (k j) -> p k j", k=C)
            nc.vector.tensor_tensor(
                ind3, s3, nd[:].unsqueeze(2).broadcast_to([PB, C, C]),
                mybir.AluOpType.is_equal)
            # index-encode in place (ind *= jencf), bf16 2x
            nc.vector.tensor_tensor(ind[:], ind[:], jencf[:],
                                    mybir.AluOpType.mult)

            nc.vector.tensor_reduce(bpbuf[:, t * C:(t + 1) * C], ind3,
                                    axis=mybir.AxisListType.X,
                                    op=mybir.AluOpType.max)

            delta_new = state.tile([PB, C], F32, tag="delta")
            nc.vector.tensor_tensor(delta_new[:], nd[:],
                                    xc[:, (t % CL) * C:(t % CL + 1) * C],
                                    mybir.AluOpType.add)
            delta = delta_new

        # ---------------- init backtrack ----------------
        m8 = state.tile([PB, 8], F32, tag="m8")
        j8 = state.tile([PB, 8], mybir.dt.uint32, tag="j8")
        nc.vector.max(m8[:], delta[:])
        nc.vector.max_index(j8[:], m8[:], delta[:])
        jf = state.tile([PB, 8], F32, tag="jf")
        nc.vector.tensor_copy(jf[:], j8[:])
        path = state.tile([PB, 1], F32, tag="path")  # encoded 64 - j
        nc.vector.tensor_scalar(path[:], jf[:, 0:1], -1.0, 64.0,
                                op0=mybir.AluOpType.mult,
                                op1=mybir.AluOpType.add)

        # ---------------- backward ----------------
        ohc = None
        for ti in range(nsteps - 1, -1, -1):
            cbase = ti // CS * CS
            tl = ti % CS
            if ti == nsteps - 1 or tl == CS - 1:
                ohc = ohout.tile([PB, CS * C], F32, tag="ohc")
                # absorb the WAR on the store that last read this slot
                nc.vector.memset(ohc[:, 0:1], 0)

            oh = ohc[:, tl * C:(tl + 1) * C]
            nc.vector.tensor_scalar(oh, jenc[:], path[:], None,
                                    op0=mybir.AluOpType.is_equal)

            if tl == 0:
                csz = min(CS, nsteps - cbase)
                # stores on virgin SP queues q0..q7 (8 chunks of 128)
                dma(nc.sync, out_d[:, cbase:cbase + csz, :],
                    ohc[:, 0:csz * C])

            if ti == 0:
                break

            bpf = state.tile([PB, C], F32, tag="bpf")
            nc.vector.tensor_copy(bpf[:], bpbuf[:, ti * C:(ti + 1) * C])
            prod = state.tile([PB, C], F32, tag="prod")
            nc.vector.tensor_tensor(prod[:], oh, bpf[:], mybir.AluOpType.mult)
            path_new = state.tile([PB, 1], F32, tag="path")
            nc.vector.tensor_reduce(path_new[:], prod[:],
                                    axis=mybir.AxisListType.X,
                                    op=mybir.AluOpType.add)
            path = path_new

        # Pre-observe every DMA queue's completion on the SP proc via one
        # single-wait nop each, so the kernel-tail drain's wait set dedups
        # to <= 1 (HW instructions carry at most one sem wait).
        for di in dma_insts:
            nop = nc.sync.nop()
            add_dep_helper(nop.ins, di.ins, sync=True, reason="tail-observe")

    return nc


_NC_CACHE = {}
LAST_EXEC_NS = None


def kernel(x: np.ndarray, transition: np.ndarray) -> np.ndarray:
    global LAST_EXEC_NS
    x = np.ascontiguousarray(x, dtype=np.float32)
    transition = np.ascontiguousarray(transition, dtype=np.float32)
    assert x.shape == (B, T, C) and transition.shape == (C, C)

    if "nc" not in _NC_CACHE:
        _NC_CACHE["nc"] = build_kernel()
    nc = _NC_CACHE["nc"]

    # constants: tkj[b, k*C + j] = trans[j, k];  jenc[b, k] = 64 - k
    tkj = np.ascontiguousarray(transition.T).reshape(1, C * C).repeat(PB, 0)
    jenc = (64.0 - np.arange(C, dtype=np.float32))[None, :].repeat(PB, 0)
    import ml_dtypes
    jencf = np.tile(64.0 - np.arange(C, dtype=np.float32), C)[None, :].repeat(PB, 0)
    jencf = np.ascontiguousarray(jencf.astype(ml_dtypes.bfloat16))

    in_maps = []
    for c in range(NCORES):
        shard = np.ascontiguousarray(x[c * PB:(c + 1) * PB])
        in_maps.append({"x": shard, "tkj": tkj.copy(), "jenc": jenc.copy(),
                        "jencf": jencf.copy()})

    res = run_bass_kernel_spmd(nc, in_maps, core_ids=list(range(NCORES)))
    LAST_EXEC_NS = res.exec_time_ns
    out = np.concatenate([res.results[c]["out"] for c in range(NCORES)], axis=0)
    return out



# revision 25
# speedup vs baseline: 1.5763x; 1.5763x over previous
"""ChainCRF Viterbi decode kernel for Trainium2 (8 NeuronCores, data parallel).

Problem: x [1024, 1024, 48] f32, transition [48, 48] f32.
Reference: per-sequence Viterbi (max-plus DP over T=1024 steps, C=48 tags,
backtrack, one-hot output [B, T, C]).

Sharding: batch 1024 -> 8 cores x 128 sequences; partition = sequence.

Lazy-backpointer scheme (exact w.r.t. the reference):
  Forward (per step, DVE only): s[b,k,j] = delta[b,j] + trans[j,k];
  nd[b,k] = max_j s; delta'[b,k] = nd + x[b,t,k].  NO backpointers are
  computed; instead every delta row is stored (blocks DMA'd to DRAM
  scratch).
  Backward reconstructs the single needed backpointer per step from the
  identity  max_j(delta_{t-1}[j] + T[j,k*]) = nd_t[k*]:
    oh      = (jenc == code)                     one-hot of k* [PB,48]
    ohT     = PE-transpose(oh)                   [48,PB] (0/1 exact)
    Tcol    = PE-matmul(lhsT=ohT, rhs=transT)    [PB,48] = T[:,k*] rows
    cand,v  = ttr(delta_{t-1} + Tcol, max)       same f32 adds as fwd
    enc     = (cand == v); code' = ttr(enc*jenc, max)   first-idx tiebreak
  code encodes 64 - j (smallest index wins ties, matching jnp.argmax).
  The one-hot select Tcol = sum_k T[j,k]*oh[k] is exact up to the PE's
  fp32 path on values |T|<=0.05 (error ~4e-7, far below the f32 ulp of
  delta ~2300 that gates every comparison).
  Backward runs as 2 interleaved sequence-groups of 64 to hide the
  DVE->PE->Pool->DVE chain latency.
"""
import sys

sys.path.insert(0, "/opt/trn_rl_repo")

from contextlib import ExitStack

import numpy as np

import bass_rust as _bass_rust

import concourse.bass as bass
import concourse.tile as tile
from concourse import mybir
from concourse.bass_utils import run_bass_kernel_spmd
from concourse.tile_rust import add_dep_helper

B, T, C = 1024, 1024, 48
NCORES = 8
PB = B // NCORES   # 128 sequences per core
CL = 128           # x-load chunk (time steps); 8 loads
W = 64             # delta-row block (steps per DRAM scratch block); 16 blocks
CS = 64            # output-store chunk; 16 stores
NG = 2             # backward seq-groups
GS = PB // NG      # 64 seqs per group
F32 = mybir.dt.float32
NEG = -3.0e38


def build_kernel(nsteps=T):
    nc = bass.Bass("TRN2", num_devices=NCORES)
    x_d = nc.dram_tensor("x", [PB, T, C], F32, kind="ExternalInput").ap()
    tkj_d = nc.dram_tensor("tkj", [PB, C * C], F32, kind="ExternalInput").ap()
    jenc_d = nc.dram_tensor("jenc", [PB, C], F32, kind="ExternalInput").ap()
    ttr_d = nc.dram_tensor("ttr", [C, C], F32, kind="ExternalInput").ap()
    ident_d = nc.dram_tensor("ident", [PB, PB], F32, kind="ExternalInput").ap()
    out_d = nc.dram_tensor("out", [PB, T, C], F32, kind="ExternalOutput").ap()
    # delta-row scratch (DRAM round trip), rows t = 0..nsteps-2 used
    nblk = (nsteps + W - 1) // W
    dsc_d = nc.dram_tensor("dscratch", [PB, nblk * W * C], F32,
                           kind="Internal").ap()

    with tile.TileContext(nc) as tc, ExitStack() as ctx:
        const = ctx.enter_context(tc.tile_pool(name="const", bufs=1))
        xin = ctx.enter_context(tc.tile_pool(name="xin", bufs=2))
        dblkp = ctx.enter_context(tc.tile_pool(name="dblk", bufs=2))
        big = ctx.enter_context(tc.tile_pool(name="big", bufs=2))
        state = ctx.enter_context(tc.tile_pool(name="state", bufs=2))

        dma_insts = []

        def dma(eng, out, in_):
            dma_insts.append(eng.dma_start(out, in_))

        def absorb(ap):
            # Absorb a DMA completion wait into a standalone in-place DVE
            # copy so compute ops never need a second sem wait.
            nc.vector.tensor_copy(ap, ap)

        tkj = const.tile([PB, C * C], F32)
        jenc = const.tile([PB, C], F32)      # jenc[k] = 64 - k
        ttrt = const.tile([C, C], F32)       # ttrt[k, j] = trans[j, k]
        ident = const.tile([PB, PB], F32)
        dma(nc.gpsimd, tkj[:], tkj_d[:])
        absorb(tkj[:, 0:1])
        dma(nc.gpsimd, jenc[:], jenc_d[:])
        absorb(jenc[:, 0:1])
        dma(nc.gpsimd, ttrt[:], ttr_d[:])
        absorb(ttrt[:, 0:1])
        dma(nc.gpsimd, ident[:], ident_d[:])
        absorb(ident[:, 0:1])
        tkj3 = tkj[:].rearrange("p (k j) -> p k j", k=C)

        # ---------------- forward ----------------
        def load_chunk(ci):
            t0 = ci * CL
            sz = min(CL, nsteps - t0)
            xc = xin.tile([PB, CL * C], F32, tag="xc", name="xc")
            eng = nc.gpsimd if ci % 2 == 0 else nc.sync
            dma(eng, xc[:, 0:sz * C], x_d[:, t0:t0 + sz, :])
            absorb(xc[:, 0:sz * C])
            return xc

        def new_dblk():
            return dblkp.tile([PB, W * C], F32, tag="dblk", name="dblk")

        xc = load_chunk(0)
        dblk = new_dblk()
        # delta_0 = x[0]
        nc.vector.tensor_copy(dblk[:, 0:C], xc[:, 0:C])
        delta = dblk[:, 0:C]

        for t in range(1, nsteps):
            if t % CL == 0:
                xc = load_chunk(t // CL)

            s = big.tile([PB, C * C], F32, tag="s")
            s3 = s[:].rearrange("p (k j) -> p k j", k=C)
            nc.vector.tensor_tensor(
                s3, delta.unsqueeze(1).broadcast_to([PB, C, C]), tkj3,
                mybir.AluOpType.add)

            nd = state.tile([PB, C], F32, tag="nd")
            nc.vector.tensor_reduce(nd[:], s3, axis=mybir.AxisListType.X,
                                    op=mybir.AluOpType.max)

            tl = t % W
            if tl == 0:
                # previous block complete -> DRAM scratch
                blk = t // W - 1
                dma(nc.sync, dsc_d[:, blk * W * C:(blk + 1) * W * C],
                    dblk[:, 0:W * C])
                dblk = new_dblk()
            nc.vector.tensor_tensor(dblk[:, tl * C:(tl + 1) * C], nd[:],
                                    xc[:, (t % CL) * C:(t % CL + 1) * C],
                                    mybir.AluOpType.add)
            delta = dblk[:, tl * C:(tl + 1) * C]

        # store the final (partial) block: rows up to nsteps-1 inclusive
        lastblk = (nsteps - 1) // W
        lastsz = (nsteps - 1) % W + 1
        dma(nc.sync, dsc_d[:, lastblk * W * C:lastblk * W * C + lastsz * C],
            dblk[:, 0:lastsz * C])

        # ---------------- seed backtrack ----------------
        m8 = state.tile([PB, 8], F32, tag="m8")
        j8 = state.tile([PB, 8], mybir.dt.uint32, tag="j8")
        nc.vector.max(m8[:], delta)
        nc.vector.max_index(j8[:], m8[:], delta)
        jf = state.tile([PB, 8], F32, tag="jf")
        nc.vector.tensor_copy(jf[:], j8[:])
        code = state.tile([PB, 1], F32, tag="code", name="code")
        nc.vector.tensor_scalar(code[:], jf[:, 0:1], -1.0, 64.0,
                                op0=mybir.AluOpType.mult,
                                op1=mybir.AluOpType.add)

        # ---------------- backward ----------------
        ohout = ctx.enter_context(tc.tile_pool(name="ohout", bufs=2))
        rdblkp = ctx.enter_context(tc.tile_pool(name="rdblk", bufs=2))
        bstate = ctx.enter_context(tc.tile_pool(name="bstate", bufs=2))
        psTp = ctx.enter_context(
            tc.tile_pool(name="psT", bufs=2, space=bass.MemorySpace.PSUM))
        psMp = ctx.enter_context(
            tc.tile_pool(name="psM", bufs=2, space=bass.MemorySpace.PSUM))
        ohTp = ctx.enter_context(tc.tile_pool(name="ohT", bufs=2))

        def load_dblk(blk, sz):
            rd = rdblkp.tile([PB, W * C], F32, tag="rdblk", name="rdblk")
            eng = nc.gpsimd if blk % 2 == 0 else nc.sync
            dma(eng, rd[:, 0:sz * C], dsc_d[:, blk * W * C:blk * W * C + sz * C])
            absorb(rd[:, 0:sz * C])
            return rd

        # backward needs delta_{t-1} for t = nsteps-1 .. 1 -> rows 0..nsteps-2
        rblk_idx = (nsteps - 2) // W
        rdblk = load_dblk(rblk_idx, (nsteps - 2) % W + 1)

        ohc = None
        for t in range(nsteps - 1, -1, -1):
            cbase = t // CS * CS
            tl = t % CS
            if t == nsteps - 1 or tl == CS - 1:
                ohc = ohout.tile([PB, CS * C], F32, tag="ohc")
                nc.vector.memset(ohc[:, 0:1], 0)

            oh = ohc[:, tl * C:(tl + 1) * C]
            nc.vector.tensor_scalar(oh, jenc[:], code[:], None,
                                    op0=mybir.AluOpType.is_equal)

            if tl == 0:
                csz = min(CS, nsteps - cbase)
                dma(nc.sync, out_d[:, cbase:cbase + csz, :],
                    ohc[:, 0:csz * C])

            if t == 0:
                break

            # delta_{t-1} row
            tp = t - 1
            if tp % W == W - 1 and tp // W != rblk_idx:
                pass  # unreachable: rblk_idx tracks current block
            if tp // W != rblk_idx:
                rblk_idx = tp // W
                rdblk = load_dblk(rblk_idx, W)
            dprev = rdblk[:, (tp % W) * C:(tp % W + 1) * C]

            # one-hot -> transposed one-hot [C, PB]
            psT = psTp.tile([C, PB], F32, tag="psT", name="psT")
            nc.tensor.transpose(psT[:], oh, ident[:])
            ohT = ohTp.tile([C, PB], F32, tag="ohT", name="ohT")
            nc.vector.tensor_copy(ohT[:], psT[:])
            # Tcol rows: psM[b, j] = trans[j, k*_b]
            psM = psMp.tile([PB, C], F32, tag="psM", name="psM")
            nc.tensor.matmul(psM[:], ohT[:], ttrt[:])
            # cand = delta_{t-1} + Tcol ; v = max_j cand
            cand = bstate.tile([PB, C], F32, tag="cand", name="cand")
            v = bstate.tile([PB, 1], F32, tag="v", name="v")
            nc.vector.tensor_tensor(cand[:], dprev, psM[:], mybir.AluOpType.add)
            nc.vector.tensor_reduce(v[:], cand[:], axis=mybir.AxisListType.X,
                                    op=mybir.AluOpType.max)
            enc = bstate.tile([PB, C], F32, tag="enc", name="enc")
            nc.vector.tensor_scalar(enc[:], cand[:], v[:], None,
                                    op0=mybir.AluOpType.is_equal)
            junk = bstate.tile([PB, C], F32, tag="junk", name="junk")
            code = bstate.tile([PB, 1], F32, tag="bcode", name="bcode")
            nc.vector.tensor_tensor(junk[:], enc[:], jenc[:],
                                    mybir.AluOpType.mult)
            nc.vector.tensor_reduce(code[:], junk[:], axis=mybir.AxisListType.X,
                                    op=mybir.AluOpType.max)

        # Pre-observe every DMA queue's completion on the SP proc via one
        # single-wait nop each, so the kernel-tail drain's wait set dedups
        # to <= 1 (HW instructions carry at most one sem wait).
        for di in dma_insts:
            nop = nc.sync.nop()
            add_dep_helper(nop.ins, di.ins, sync=True, reason="tail-observe")

    # Legalize to the HW constraint of <=1 sem wait per instruction
    # (EventSemaphore carries the overflow) -- the passes Bacc.compile runs.
    _bass_rust.move_matmul_waits_to_ldweights(nc.m)
    _bass_rust.generate_event_semaphores(nc)
    mybir.codegen_inst_isa_subclasses(nc)
    return nc


_NC_CACHE = {}
LAST_EXEC_NS = None


def kernel(x: np.ndarray, transition: np.ndarray) -> np.ndarray:
    global LAST_EXEC_NS
    x = np.ascontiguousarray(x, dtype=np.float32)
    transition = np.ascontiguousarray(transition, dtype=np.float32)
    assert x.shape == (B, T, C) and transition.shape == (C, C)

    if "nc" not in _NC_CACHE:
        import os
        ns = int(os.environ.get("KNSTEPS", T))
        _NC_CACHE["nc"] = build_kernel(nsteps=ns)
    nc = _NC_CACHE["nc"]

    # constants: tkj[b, k*C + j] = trans[j, k];  jenc[b, k] = 64 - k
    tkj = np.ascontiguousarray(transition.T).reshape(1, C * C).repeat(PB, 0)
    jenc = (64.0 - np.arange(C, dtype=np.float32))[None, :].repeat(PB, 0)
    ttr = np.ascontiguousarray(transition.T)          # ttr[k, j] = trans[j, k]
    ident = np.eye(PB, dtype=np.float32)

    in_maps = []
    for c in range(NCORES):
        shard = np.ascontiguousarray(x[c * PB:(c + 1) * PB])
        in_maps.append({"x": shard, "tkj": tkj.copy(), "jenc": jenc.copy(),
                        "ttr": ttr.copy(), "ident": ident.copy()})

    res = run_bass_kernel_spmd(nc, in_maps, core_ids=list(range(NCORES)))
    LAST_EXEC_NS = res.exec_time_ns
    out = np.concatenate([res.results[c]["out"] for c in range(NCORES)], axis=0)
    return out
